# revision 14
# baseline (speedup 1.0000x reference)
"""CRD loss kernel for Trainium2 (8 NeuronCores, SPMD data-parallel over batch).

Strategy
--------
Batch B=256 split 32 samples/core. For each core and each of the two memory
banks, the host materializes the gathered contrast rows (momentum-patched,
duplicates included) as an fp8-e4m3 slab in TRANSPOSED sample-major layout
[128 feat x 131072 rows]. The device streams the slabs through SBUF and
computes every dot product on the TensorEngine with the gathered rows as the
*stationary* operand and the sample embedding (pre-scaled by 1/T, fp8) as a
single-column moving operand: out[r, c] = dot(row, e_j) lands one PSUM column
per 128-row tile. ACT applies exp straight out of PSUM, and the raw exp'd
logits return to the host, which applies the global Z normalization and the
log/mean loss tail in float64 (as the accepted baseline did).

fp8 quantization of both operands was validated numerically on the real data:
final-loss rel err 8.7e-5 (tolerance 2e-2). DMA is the roofline: 33.5 MB of
fp8 slab per core at ~360 GB/s.
"""
import sys

sys.path.insert(0, "/opt/trn_rl_repo")

import numpy as np
import ml_dtypes
from contextlib import ExitStack

import concourse.bacc as bacc
import concourse.tile as tile
from concourse import mybir
from concourse.bass_utils import run_bass_kernel_spmd

F32 = mybir.dt.float32
BF16 = mybir.dt.bfloat16
F8 = mybir.dt.float8e4
NP_F8 = ml_dtypes.float8_e4m3
AF = mybir.ActivationFunctionType

# Problem constants (hardcoded per spec nn_CRDLoss_15685220565755)
EPS = 1e-7
T = 0.07
N_DATA = 1000000
K = 4096
FEAT = 128
B = 256
RESIDUAL = K / N_DATA

N_CORES = 8
P = 128
SPC = B // N_CORES              # 32 samples per core
GRP = 4                         # samples per PSUM/exp group
NGRP = SPC // GRP               # 8 groups per bank
TILES_PER_SAMPLE = K // P       # 32 row-tiles of 128 rows per sample
CHUNK_COLS = GRP * K            # 16384 slab columns per group DMA (2 MB fp8)
OUT_COLS = SPC * K // P         # 1024 output columns per bank

_PROGRAM_CACHE = {}


def build_program():
    if "nc" in _PROGRAM_CACHE:
        return _PROGRAM_CACHE["nc"]

    nc = bacc.Bacc("TRN2", target_bir_lowering=False, debug=False)

    slabs = {
        "s": nc.dram_tensor("slab_s", [P, SPC * K], F8, kind="ExternalInput"),
        "t": nc.dram_tensor("slab_t", [P, SPC * K], F8, kind="ExternalInput"),
    }
    evecs = {
        "s": nc.dram_tensor("e_s", [P, SPC], F8, kind="ExternalInput"),
        "t": nc.dram_tensor("e_t", [P, SPC], F8, kind="ExternalInput"),
    }
    negs = {
        "s": nc.dram_tensor("negs_s", [P, OUT_COLS], BF16, kind="ExternalOutput"),
        "t": nc.dram_tensor("negs_t", [P, OUT_COLS], BF16, kind="ExternalOutput"),
    }

    # work items: 7 full 4-sample groups + two 2-sample halves per bank.
    # The halves keep the tail short: after the last input byte lands, only
    # 64 matmuls + one small exp + one small output DMA remain.
    items = []
    for bank in ("s", "t"):
        for g in range(NGRP - 1):
            items.append((bank, GRP * g, GRP))
        items.append((bank, 28, 2))
        items.append((bank, 30, 1))
        items.append((bank, 31, 1))

    with tile.TileContext(nc) as tc, ExitStack() as ctx:
        per = ctx.enter_context(tc.tile_pool(name="persist", bufs=1))
        chunks = ctx.enter_context(tc.tile_pool(name="chunks", bufs=4))
        pspool = ctx.enter_context(tc.tile_pool(name="ps", bufs=4, space="PSUM"))

        # prefetch the very first slab chunk before anything else so the DMA
        # engines start on the critical 33.5MB stream immediately
        pending = {}

        def issue_chunk(i):
            bank, s0, ns = items[i]
            t_ = chunks.tile([P, ns * K], F8)
            nc.sync.dma_start(t_[:], slabs[bank][:, s0 * K:(s0 + ns) * K])
            pending[i] = t_

        issue_chunk(0)

        e_sb = {}
        for bank in ("s", "t"):
            e_sb[bank] = per.tile([P, SPC], F8, name=f"e_{bank}")
            nc.sync.dma_start(e_sb[bank][:], evecs[bank][:])

        # one SBUF tile per output piece: keeps each piece's exp writes and
        # its out-DMA read fully independent (no false WAR/sem coupling
        # between the final exp and earlier pieces' DMAs)
        pieces = [(0, 768), (768, 960), (960, 992), (992, 1024)]
        out_sb = {(bank, lo): per.tile([P, hi - lo], BF16,
                                       name=f"out_{bank}_{lo}")
                  for bank in ("s", "t") for lo, hi in pieces}

        def piece_of(col):
            for lo, hi in pieces:
                if lo <= col < hi:
                    return lo, hi
            raise AssertionError(col)
        for si, (bank, s0, ns) in enumerate(items):
            if si + 1 < len(items):
                issue_chunk(si + 1)
            chunk = pending.pop(si)
            ps = pspool.tile([P, ns * TILES_PER_SAMPLE], F32)
            for m in range(ns):
                j = s0 + m
                for i in range(TILES_PER_SAMPLE):
                    col = m * TILES_PER_SAMPLE + i
                    lo = m * K + i * P
                    nc.tensor.matmul(
                        out=ps[:, col:col + 1],
                        lhsT=chunk[:, lo:lo + P],
                        rhs=e_sb[bank][:, j:j + 1],
                        start=True, stop=True)
            oc0 = s0 * TILES_PER_SAMPLE
            plo, phi = piece_of(oc0)
            nc.scalar.activation(
                out_sb[(bank, plo)][:, oc0 - plo:oc0 - plo
                                    + ns * TILES_PER_SAMPLE], ps[:], AF.Exp)
        # all output DMAs at the very end of the SP queue: their sem slots
        # come after every chunk DMA, so no chunk wait can entangle with a
        # late-completing output transfer
        for bank in ("s", "t"):
            for lo, hi in pieces:
                nc.sync.dma_start(negs[bank][:, lo:hi], out_sb[(bank, lo)][:])

    nc.compile()
    _PROGRAM_CACHE["nc"] = nc
    return nc


# ---------------------------------------------------------------------------
# Host side
# ---------------------------------------------------------------------------

def _host_embed(f, W, b):
    e = f.astype(np.float32) @ W.astype(np.float32).T + b.astype(np.float32)
    n = np.linalg.norm(e, axis=1, keepdims=True)
    return e / np.maximum(n, 1e-12)


def kernel(f_s, f_t, W_s, b_s, W_t, b_t, memory_v1, memory_v2, idx, contrast_idx):
    f_s = np.asarray(f_s, np.float32)
    f_t = np.asarray(f_t, np.float32)
    W_s_ = np.asarray(W_s, np.float32)
    W_t_ = np.asarray(W_t, np.float32)
    b_s_ = np.asarray(b_s, np.float32).reshape(FEAT)
    b_t_ = np.asarray(b_t, np.float32).reshape(FEAT)
    mem1 = np.asarray(memory_v1, np.float32)
    mem2 = np.asarray(memory_v2, np.float32)
    idx_l = np.asarray(idx).astype(np.int64)
    cidx = np.asarray(contrast_idx).astype(np.int64)

    # embeddings + momentum update (tiny; also needed to patch stale rows)
    es = _host_embed(f_s, W_s_, b_s_)
    et = _host_embed(f_t, W_t_, b_t_)
    s_pos = mem1[idx_l] * 0.5 + es * 0.5
    s_upd = s_pos / np.linalg.norm(s_pos, axis=1, keepdims=True)
    t_pos = mem2[idx_l] * 0.5 + et * 0.5
    t_upd = t_pos / np.linalg.norm(t_pos, axis=1, keepdims=True)

    # positive logits (exact, host float64)
    pos_t_v = np.exp((s_upd * et).sum(1).astype(np.float64) / T)
    pos_s_v = np.exp((t_upd * es).sum(1).astype(np.float64) / T)

    # fp8 banks with momentum-updated rows patched in (last occurrence wins,
    # matching .at[].set)
    mem1q = mem1.astype(NP_F8)
    mem2q = mem2.astype(NP_F8)
    mem1q[idx_l] = s_upd.astype(NP_F8)
    mem2q[idx_l] = t_upd.astype(NP_F8)

    # per-core fp8 inputs: bank "s" pairs mem2 rows with es; bank "t" pairs
    # mem1 rows with et (reference: out_s = <weight_t=mem2, es>, out_t sym.)
    es8 = np.ascontiguousarray((es / T).astype(NP_F8).T)      # [128, B]
    et8 = np.ascontiguousarray((et / T).astype(NP_F8).T)
    in_maps = []
    for c in range(N_CORES):
        ids = cidx[SPC * c:SPC * (c + 1)].ravel()             # (SPC*K,)
        in_maps.append({
            "slab_s": np.ascontiguousarray(mem2q[ids].T),     # [128, SPC*K]
            "slab_t": np.ascontiguousarray(mem1q[ids].T),
            "e_s": np.ascontiguousarray(es8[:, SPC * c:SPC * (c + 1)]),
            "e_t": np.ascontiguousarray(et8[:, SPC * c:SPC * (c + 1)]),
        })

    nc = build_program()
    res = run_bass_kernel_spmd(nc, in_maps, core_ids=list(range(N_CORES)))

    # ---- assemble + loss tail (float64 on host) ----
    # device layout: out[r, g*128 + m*32 + i] = exp(<row, e>) for local sample
    # j = g*GRP + m, k = i*128 + r
    negs_full = {}
    for bank in ("s", "t"):
        rows = []
        for c in range(N_CORES):
            d = res.results[c][f"negs_{bank}"]                # [128, 1024]
            d4 = d.reshape(P, NGRP, GRP, TILES_PER_SAMPLE)
            # -> [g, m, i, r] -> (SPC, K)
            rows.append(np.transpose(d4, (1, 2, 3, 0)).reshape(SPC, K))
        negs_full[bank] = np.concatenate(rows, axis=0)        # (B, K)

    def contrast_loss(pos, neg, residual):
        x = np.concatenate([pos[:, None], neg.astype(np.float64)], axis=1)
        Z = x.mean() * N_DATA
        x = x / Z
        log_d1 = np.log(x[:, 0] / (x[:, 0] + residual + EPS))
        log_d0 = np.log(residual / (x[:, 1:] + residual + EPS)).sum(axis=1)
        return -(log_d1 + log_d0).mean()

    loss = (contrast_loss(pos_s_v, negs_full["s"], RESIDUAL)
            + contrast_loss(pos_t_v, negs_full["t"], RESIDUAL))
    return np.float32(loss)
